# revision 31
# baseline (speedup 1.0000x reference)
"""Trainium2 Bass kernel for nn_AdaptivePiecewiseLinear.

Math: for each (b, j):  y[b, j] = sum_i interp(values[i, j, :], t[b, i])
where t = wrap(x) mapped to knot coordinates [0, NP-1).

Piecewise-linear interpolation on a uniform grid == matmul with a hat-basis
matrix:  y = M @ V,  M[b, (k,i)] = hat(t[b,i] - k),  V[(k,i), j] = values[i,j,k],
hat(u) = relu(1 - |u|).

Sharding: contraction dim i is split 8 ways (32 input features per core).
Each core builds only its [2048, 512] slice of M^T on DVE/ACT, runs 32
bf16 matmuls on TensorE accumulating y^T [256, 512] in PSUM, then a
ReduceScatter(add) over the 8 cores leaves each core with its 32-row
j-shard, DMA'd out. Host assembles shards and transposes.
"""

import numpy as np
import ml_dtypes

import concourse.bass as bass
import concourse.mybir as mybir
import concourse.tile as tile
from concourse import bacc
from concourse.bass_utils import run_bass_kernel_spmd

B = 512
NI = 256
NO = 256
NP = 64
W = 8                 # cores
NI_SH = NI // W       # 32 input features per core
IK = NI_SH * NP       # 2048 contraction length per core
NT = IK // 128        # 16 contraction tiles
JT = NO // 128        # 2 psum j-halves
REP = 128 // NI_SH    # 4 partition replicas of x

# Logical rank -> physical NC id on the chip (trn2 driver nc remap), and the
# chip's own routing id, used by the rdma exchange mode.
PID_MAP = (0, 1, 2, 3, 6, 7, 4, 5)
RID = 0


def build_kernel(scale: float, offset: float, mode: str = "rs"):
    """Build the SPMD Bass graph (same on all 8 cores).

    t = frac(x*scale + offset) * (NP-1) maps wrapped x into knot coords.
    offset includes +64 so the mod operand is always positive.
    mode: "rs" = on-device ReduceScatter, each core outputs its j-shard.
          "partial" = no collective; each core outputs its full partial sum.
    """
    nc = bacc.Bacc("TRN2", target_bir_lowering=False, debug=False, num_devices=W)

    xt = nc.dram_tensor("xt", [NI_SH, B], mybir.dt.float32, kind="ExternalInput")
    v2 = nc.dram_tensor("v2", [IK, NO], mybir.dt.bfloat16, kind="ExternalInput")
    kb = nc.dram_tensor("kb", [128, NT], mybir.dt.float32, kind="ExternalInput")
    kb1 = nc.dram_tensor("kb1", [128, NT], mybir.dt.float32, kind="ExternalInput")
    kb2 = nc.dram_tensor("kb2", [128, NT], mybir.dt.float32, kind="ExternalInput")
    if mode == "rs":
        out_shape = [NO // W, B]
    elif mode == "rdma":
        out_shape = [128, 128]
    else:
        out_shape = [NO, B]
    out = nc.dram_tensor("out", out_shape, mybir.dt.bfloat16, kind="ExternalOutput")

    AF = mybir.ActivationFunctionType
    OP = mybir.AluOpType

    with tile.TileContext(nc) as tc:
        with (
            tc.tile_pool(name="sb", bufs=1) as sb,
            tc.tile_pool(name="mp", bufs=4) as mp,
            tc.tile_pool(name="ps", bufs=1, space="PSUM") as ps,
            tc.tile_pool(name="dram", bufs=1, space="DRAM") as dp,
        ):
            # --- loads ---
            # Two DMAs for the 16 contraction tiles of V2 (dst[p, T, j] =
            # v2[128T + p, j]); the first half lands early so matmuls can
            # start while the second half streams in.
            vt_all = sb.tile([128, NT * NO], mybir.dt.bfloat16, tag="vt")
            H = NT // 2
            vtv = vt_all[:].rearrange("p (t j) -> p t j", t=NT)
            v2v = v2.rearrange("(t p) j -> p t j", p=128)
            nc.sync.dma_start(out=vtv[:, 0:H, :], in_=v2v[:, 0:H, :])
            nc.sync.dma_start(out=vtv[:, H:NT, :], in_=v2v[:, H:NT, :])
            vt = [vt_all[:, T * NO:(T + 1) * NO] for T in range(NT)]

            # HWDGE engines only (sync/scalar) -- gpsimd SWDGE desc-gen costs
            # ~5us and would gate the prep chain.
            xr = sb.tile([128, B], mybir.dt.float32, tag="xr")
            xeng = [nc.scalar, nc.sync, nc.scalar, nc.sync]
            for r in range(REP):
                xeng[r % len(xeng)].dma_start(
                    out=xr[r * NI_SH:(r + 1) * NI_SH, :], in_=xt[:, :]
                )
            kbs = sb.tile([128, NT], mybir.dt.float32, tag="kb")
            nc.scalar.dma_start(out=kbs[:], in_=kb[:, :])
            kb1s = sb.tile([128, NT], mybir.dt.float32, tag="kb1")
            nc.scalar.dma_start(out=kb1s[:], in_=kb1[:, :])
            kb2s = sb.tile([128, NT], mybir.dt.float32, tag="kb2")
            nc.scalar.dma_start(out=kb2s[:], in_=kb2[:, :])

            # Pull the ACT Abs table load off the critical path: a 1-element
            # dummy Abs right after the kb DMA completes.
            tdmy = sb.tile([128, 1], mybir.dt.float32, tag="tdmy")
            nc.scalar.activation(tdmy[:], kbs[:, 0:1], AF.Abs, bias=0.0, scale=1.0)

            # --- prep: f[p,b] = frac(x*scale + offset) = wrapped pos in [0,1) ---
            # q in [61,68]; floor via the +2^23 round trick (q-0.5 rounded to
            # nearest) -- valid because q is positive and << 2^22.
            q = sb.tile([128, B], mybir.dt.float32, tag="q")
            nc.vector.tensor_scalar(q[:], xr[:], scale, offset, OP.mult, OP.add)
            r = sb.tile([128, B], mybir.dt.float32, tag="r")
            nc.vector.tensor_scalar(
                r[:], q[:], float(2**23) - 0.5, float(2**23), OP.add, OP.subtract
            )
            f = sb.tile([128, B], mybir.dt.float32, tag="f")
            nc.vector.tensor_sub(f[:], q[:], r[:])

            # --- main pipeline: M-tile build + matmul accumulate ---
            # u = |63*f + kb[:,T]|  (kb[p,T] = -(4T + p>>5));  m = min(u-1, 0)
            # = -hat. The negation is undone in the psum->sbuf copy (scale=-1).
            pst = [
                ps.tile([128, B], mybir.dt.float32, tag=f"ps{j}", name=f"ps{j}")
                for j in range(JT)
            ]
            # A few tiles take a pure-DVE path (2x tensor_scalar + max + min)
            # to offload the ACT Abs chain, which is otherwise critical.
            DVE_TILES = {5, 10, 15}
            for T in range(NT):
                m = mp.tile([128, B], mybir.dt.bfloat16, tag="m", name=f"m{T}")
                if T in DVE_TILES:
                    d1 = mp.tile([128, B], mybir.dt.bfloat16, tag="d1", name=f"d1{T}")
                    nc.vector.tensor_scalar(
                        d1[:], f[:], float(NP - 1), kb1s[:, T:T + 1], OP.mult, OP.add
                    )
                    d2 = mp.tile([128, B], mybir.dt.bfloat16, tag="d2", name=f"d2{T}")
                    nc.vector.tensor_scalar(
                        d2[:], f[:], float(1 - NP), kb2s[:, T:T + 1], OP.mult, OP.add
                    )
                    mx = mp.tile([128, B], mybir.dt.bfloat16, tag="mx", name=f"mx{T}")
                    nc.vector.tensor_max(mx[:], d1[:], d2[:])
                    nc.vector.tensor_scalar_min(m[:], mx[:], 0.0)
                else:
                    u = mp.tile([128, B], mybir.dt.bfloat16, tag="u", name=f"u{T}")
                    nc.scalar.activation(
                        u[:], f[:], AF.Abs, bias=kbs[:, T:T + 1], scale=float(NP - 1)
                    )
                    nc.vector.tensor_scalar(m[:], u[:], 1.0, 0.0, OP.subtract, OP.min)
                for j in range(JT):
                    nc.tensor.matmul(
                        pst[j][:],
                        lhsT=vt[T][:, j * 128:(j + 1) * 128],
                        rhs=m[:],
                        start=(T == 0),
                        stop=(T == NT - 1),
                    )

            # --- psum -> sbuf (negating) -> dram, ReduceScatter, out ---
            if mode == "rs":
                cc_in = dp.tile([NO, B], mybir.dt.bfloat16)
                cc_out = dp.tile([NO // W, B], mybir.dt.bfloat16)
                for j in range(JT):
                    yb = sb.tile(
                        [128, B], mybir.dt.bfloat16, tag=f"yb{j}", name=f"yb{j}"
                    )
                    nc.scalar.mul(yb[:], pst[j][:], -1.0)
                    nc.sync.dma_start(out=cc_in[j * 128:(j + 1) * 128, :], in_=yb[:])
                nc.gpsimd.collective_compute(
                    "ReduceScatter",
                    OP.add,
                    replica_groups=[list(range(W))],
                    ins=[cc_in.opt()],
                    outs=[cc_out.opt()],
                )
                nc.sync.dma_start(out=out[:, :], in_=cc_out[:])
            elif mode == "rdma":
                # DIY reduce-scatter over point-to-point remote_dma (the ncfw
                # collective has a ~60us fixed bootstrap). Scatter along B:
                # dest core s owns b-range [64s, 64s+64).
                #   yb_all[p, s*128 + jh*64 + w] = y[jh*128 + p, 64s + w]
                # Each core sends slice s -> core s's recv slot <my rank>;
                # every core then sums its 8 received slots.
                yb_all = sb.tile([128, W * 128], mybir.dt.bfloat16, tag="yball")
                ybv = yb_all[:].rearrange("p (s c) -> p s c", s=W)
                for jh in range(JT):
                    nc.scalar.mul(
                        ybv[:, :, jh * 64:(jh + 1) * 64],
                        pst[jh][:].rearrange("p (s w) -> p s w", s=W),
                        -1.0,
                    )
                recv = sb.tile([128, W * 128], mybir.dt.bfloat16, tag="recv")
                acc = sb.tile([128, 128], mybir.dt.bfloat16, tag="acc")
                rsem = nc.alloc_semaphore("rdma_recv")
                lsem = nc.alloc_semaphore("rdma_local")
                MASK = 0xF0F0          # intra-chip valid for same- and cross-die
                with tc.tile_critical():
                    off = nc.gpsimd.partition_id() * 128
                    for s in range(W):
                        nc.gpsimd.remote_dma(
                            out_ap=recv[:, bass.ds(off, 128)],
                            in_ap=yb_all[:, s * 128:(s + 1) * 128],
                            remote_sem=rsem,
                            local_sem=lsem,
                            pid=PID_MAP[s],
                            routing_id=RID,
                            dma_engine_mask=MASK,
                        )
                    nc.gpsimd.trigger_dma(count=None)
                    nc.vector.wait_ge(rsem, W * bin(MASK).count("1"))
                    rv = recv[:].rearrange("p (s c) -> p s c", s=W)
                    nc.vector.tensor_add(acc[:], rv[:, 0, :], rv[:, 1, :])
                    for s in range(2, W):
                        nc.vector.tensor_add(acc[:], acc[:], rv[:, s, :])
                nc.sync.dma_start(out=out[:, :], in_=acc[:])
            else:
                oeng = [nc.sync, nc.scalar]
                for j in range(JT):
                    yb = sb.tile(
                        [128, B], mybir.dt.bfloat16, tag=f"yb{j}", name=f"yb{j}"
                    )
                    if j == 0:
                        # DVE does this copy so the two psum evacuations run
                        # on different engines concurrently.
                        nc.vector.tensor_scalar(
                            yb[:], pst[j][:], -1.0, None, OP.mult
                        )
                    else:
                        nc.scalar.mul(yb[:], pst[j][:], -1.0)
                    oeng[j % 2].dma_start(
                        out=out[j * 128:(j + 1) * 128, :], in_=yb[:]
                    )
    nc.compile()
    return nc


_cached = {}

MODE = "partial"


def _get_kernel(scale, offset, mode):
    key = (scale, offset, mode)
    if key not in _cached:
        _cached[key] = build_kernel(scale, offset, mode)
    return _cached[key]


def make_in_maps(x, positions, values):
    pos_min = float(positions[0, 0, 0])
    pos_max = float(positions[0, 0, -1])
    period = pos_max - pos_min
    scale = 1.0 / period
    offset = -pos_min / period + 64.0

    # kb[p, T] = -(4T + p//NI_SH): the negated knot index handled by
    # partition p of contraction tile T.
    prow = np.repeat(np.arange(REP, dtype=np.float32), NI_SH)       # [128]
    kbmat = -(prow[:, None] + 4.0 * np.arange(NT, dtype=np.float32)[None, :])
    kbmat = np.ascontiguousarray(kbmat, dtype=np.float32)           # [128, NT]
    kb1mat = np.ascontiguousarray(kbmat - 1.0, dtype=np.float32)    # -(k+1)
    kb2mat = np.ascontiguousarray(-kbmat - 1.0, dtype=np.float32)   # k-1
    in_maps = []
    for c in range(W):
        sl = slice(c * NI_SH, (c + 1) * NI_SH)
        xt = np.ascontiguousarray(x[:, sl].T, dtype=np.float32)
        # V2 rows ordered (k major, i minor): row 32*k + i  ->  values[i, j, k]
        v2 = np.ascontiguousarray(
            values[sl].transpose(2, 0, 1).reshape(IK, NO)
        ).astype(ml_dtypes.bfloat16)
        in_maps.append({"xt": xt, "v2": v2, "kb": kbmat, "kb1": kb1mat, "kb2": kb2mat})
    return in_maps, scale, offset


def kernel(x, positions, values, _trace=False):
    in_maps, scale, offset = make_in_maps(x, positions, values)
    nc = _get_kernel(scale, offset, MODE)
    res = run_bass_kernel_spmd(nc, in_maps, core_ids=list(range(W)), trace=_trace)
    outs = [np.asarray(res.results[c]["out"]) for c in range(W)]
    if MODE == "rs":
        yT = np.concatenate(outs, axis=0)                    # [256, 512]
    elif MODE == "rdma":
        # out_s[p, jh*64 + w] = yT[jh*128 + p, 64s + w]
        yT = np.empty((NO, B), dtype=np.float32)
        for s in range(W):
            z = outs[s].astype(np.float32).reshape(128, JT, 64)
            for jh in range(JT):
                yT[jh * 128:(jh + 1) * 128, 64 * s:64 * s + 64] = z[:, jh, :]
    else:
        yT = np.sum(np.stack(outs).astype(np.float32), axis=0)
    y = np.ascontiguousarray(yT.T).astype(np.float32)        # [512, 256]
    if _trace:
        return y, res
    return y
